# revision 6
# baseline (speedup 1.0000x reference)
"""CMSBlockLinear block-ELL sparse linear forward on 8 trn2 NeuronCores.

Strategy: the block-sparse weight (R=128 x K=32 active 16x16 tiles, 25%
density) is densified on the host into W^T [2048 in, 2048 out] and cast to
bf16.  The device then runs a dense matmul y^T = W^T.T @ x^T with fp32 PSUM
accumulation.  Dense-ifying costs 4x the weight FLOPs on paper, but the PE
streams N columns per matmul regardless of M, so a dense 128-wide M uses the
array 8x better than the natural M=16 sparse formulation - dense wins on both
PE time and (with bf16) roughly matches sparse fp32 on DMA bytes.

Sharding (8 cores): 4-way over tokens x 2-way over output features.
Per core: x^T shard [2048, 512] bf16 (1 MB), W^T half [2048, 1024] bf16
(4 MB), out [1024, 512] fp32 (2 MB).

Device loop: k-outer over 16 contraction chunks of 128; per chunk one
256 KB W DMA + one 128 KB x DMA, then 8 back-to-back matmuls accumulating
into all 8 PSUM banks (one per 128-wide output chunk).  PE stays warm the
whole kernel; DMA (~17 us) hides under PE (~27 us).
"""

import os

import numpy as np

BATCH, SEQ = 4, 512
IN_F = OUT_F = 2048
B = 16
R = 128  # output block rows
C = 128  # input block cols
KBLK = 32  # active tiles per row

TOK = BATCH * SEQ  # 2048 tokens
TOK_SHARDS = 4
OUT_SHARDS = 2
TOK_PER = TOK // TOK_SHARDS  # 512
OUT_PER = OUT_F // OUT_SHARDS  # 1024
K_CHUNKS = IN_F // 128  # 16
M_CHUNKS = OUT_PER // 128  # 8

LAST_EXEC_TIME_NS = None

_CACHE = {}


def _ensure_profile_hook():
    """Provide antenv.axon_hooks if the image lacks it, so trace=True works.

    Mirrors trn_agent_boot._ntff_profile_via_ctypes: drives NTFF capture via
    the libaxon_pjrt.so C ABI.  Also makes upload_artifacts fall back to the
    local dir when no artifact store is reachable.
    """
    import contextlib
    import ctypes
    import sys
    import types

    try:
        import antenv.axon_hooks  # noqa: F401

        return
    except ImportError:
        pass

    so_path = "/opt/axon/libaxon_pjrt.so"
    _hook = None
    if os.path.exists(so_path):
        try:
            lib = ctypes.CDLL(so_path)
            if hasattr(lib, "axon_start_nrt_profile"):
                lib.axon_start_nrt_profile.argtypes = [
                    ctypes.POINTER(ctypes.c_int64),
                    ctypes.c_size_t,
                ]
                lib.axon_start_nrt_profile.restype = ctypes.c_int64
                lib.axon_stop_nrt_profile.argtypes = [ctypes.c_char_p]
                lib.axon_stop_nrt_profile.restype = ctypes.c_int64

                @contextlib.contextmanager
                def _ntff_hook(output_dir, device_ids):
                    import jax

                    jax.devices()
                    if device_ids:
                        ids = (ctypes.c_int64 * len(device_ids))(*device_ids)
                        rc = lib.axon_start_nrt_profile(ids, len(device_ids))
                    else:
                        rc = lib.axon_start_nrt_profile(None, 0)
                    if rc != 0:
                        raise RuntimeError(f"axon_start_nrt_profile rc={rc}")
                    try:
                        yield
                    finally:
                        n = lib.axon_stop_nrt_profile(str(output_dir).encode())
                        print(f"profile: {n} file(s) -> {output_dir}", file=sys.stderr)

                _hook = _ntff_hook
        except OSError:
            pass

    mod = types.ModuleType("antenv.axon_hooks")
    mod.get_axon_ntff_profile_hook = lambda: _hook
    sys.modules["antenv.axon_hooks"] = mod

    import concourse.bass_utils as _bu

    _orig_upload = _bu.upload_artifacts

    def _safe_upload(tmpdir):
        try:
            return _orig_upload(tmpdir)
        except Exception:
            return tmpdir

    _bu.upload_artifacts = _safe_upload


def _build_nc():
    import concourse.mybir as mybir
    from concourse import bacc
    from concourse.tile import TileContext

    nc = bacc.Bacc("TRN2", target_bir_lowering=False)
    xT = nc.dram_tensor("xT", [IN_F, TOK_PER], mybir.dt.bfloat16, kind="ExternalInput")
    w = nc.dram_tensor("w", [IN_F, OUT_PER], mybir.dt.bfloat16, kind="ExternalInput")
    bias = nc.dram_tensor("bias", [OUT_PER], mybir.dt.float32, kind="ExternalInput")
    y = nc.dram_tensor("y", [OUT_PER, TOK_PER], mybir.dt.float32, kind="ExternalOutput")

    with TileContext(nc) as tc:
        with (
            tc.tile_pool(name="consts", bufs=1) as consts,
            tc.tile_pool(name="xp", bufs=K_CHUNKS) as xp,
            tc.tile_pool(name="wp", bufs=K_CHUNKS) as wp,
            tc.tile_pool(name="op", bufs=2) as op,
            tc.tile_pool(name="ps", bufs=1, space="PSUM") as ps,
        ):
            bias_sb = consts.tile([128, M_CHUNKS], mybir.dt.float32)
            nc.sync.dma_start(bias_sb[:], bias.rearrange("(m p) -> p m", p=128))

            psums = [
                ps.tile([128, TOK_PER], mybir.dt.float32, tag=f"ps{m}", name=f"ps{m}")
                for m in range(M_CHUNKS)
            ]

            for k in range(K_CHUNKS):
                xk = xp.tile([128, TOK_PER], mybir.dt.bfloat16)
                nc.sync.dma_start(xk[:], xT[k * 128 : (k + 1) * 128, :])
                wk = wp.tile([128, OUT_PER], mybir.dt.bfloat16)
                nc.sync.dma_start(wk[:], w[k * 128 : (k + 1) * 128, :])
                for m in range(M_CHUNKS):
                    nc.tensor.matmul(
                        psums[m][:],
                        wk[:, m * 128 : (m + 1) * 128],
                        xk[:],
                        start=(k == 0),
                        stop=(k == K_CHUNKS - 1),
                    )

            for m in range(M_CHUNKS):
                out_sb = op.tile([128, TOK_PER], mybir.dt.float32)
                nc.vector.tensor_scalar_add(
                    out_sb[:], psums[m][:], bias_sb[:, m : m + 1]
                )
                nc.sync.dma_start(y[m * 128 : (m + 1) * 128, :], out_sb[:])

    nc.finalize()
    return nc


def _densify_wT(values: np.ndarray, col_indices: np.ndarray) -> np.ndarray:
    """W^T [in=2048, out=2048] with W[r*16+i, c*16+j] = values[r,k,i,j]."""
    wT = np.zeros((C, B, R, B), dtype=np.float32)  # [c, j, r, i]
    vals_t = values.transpose(0, 1, 3, 2)  # [R, K, j, i]
    r_idx = np.arange(R)
    wT[col_indices, :, r_idx[:, None], :] = vals_t
    return wT.reshape(IN_F, OUT_F)


def kernel(x, values, col_indices, bias):
    global LAST_EXEC_TIME_NS
    import ml_dtypes

    _ensure_profile_hook()
    from concourse.bass_utils import run_bass_kernel_spmd

    if "nc" not in _CACHE:
        _CACHE["nc"] = _build_nc()
    nc = _CACHE["nc"]

    bf16 = ml_dtypes.bfloat16
    wT = _densify_wT(np.asarray(values), np.asarray(col_indices)).astype(bf16)
    xT = np.ascontiguousarray(
        np.asarray(x, dtype=np.float32).reshape(TOK, IN_F).T
    ).astype(bf16)
    bias_f = np.asarray(bias, dtype=np.float32)

    in_maps = []
    for core in range(8):
        t, h = divmod(core, OUT_SHARDS)
        in_maps.append(
            {
                "xT": np.ascontiguousarray(xT[:, t * TOK_PER : (t + 1) * TOK_PER]),
                "w": np.ascontiguousarray(wT[:, h * OUT_PER : (h + 1) * OUT_PER]),
                "bias": np.ascontiguousarray(bias_f[h * OUT_PER : (h + 1) * OUT_PER]),
            }
        )

    res = run_bass_kernel_spmd(
        nc,
        in_maps,
        list(range(8)),
        trace=bool(os.environ.get("BASS_TRACE")),
    )
    LAST_EXEC_TIME_NS = res.exec_time_ns

    y = np.empty((TOK, OUT_F), dtype=np.float32)
    for core in range(8):
        t, h = divmod(core, OUT_SHARDS)
        y[t * TOK_PER : (t + 1) * TOK_PER, h * OUT_PER : (h + 1) * OUT_PER] = res.results[
            core
        ]["y"].T
    return y.reshape(BATCH, SEQ, OUT_F)


# revision 7
# speedup vs baseline: 1.0449x; 1.0449x over previous
"""CMSBlockLinear block-ELL sparse linear forward on 8 trn2 NeuronCores.

Strategy: the block-sparse weight (R=128 x K=32 active 16x16 tiles, 25%
density) is densified on the host into W^T [2048 in, 2048 out] and cast to
bf16.  The device then runs a dense matmul y^T = W^T.T @ x^T with fp32 PSUM
accumulation.  Dense-ifying costs 4x the weight FLOPs on paper, but the PE
streams N columns per matmul regardless of M, so a dense 128-wide M uses the
array 8x better than the natural M=16 sparse formulation - dense wins on both
PE time and (with bf16) roughly matches sparse fp32 on DMA bytes.

Sharding (8 cores): 4-way over tokens x 2-way over output features.
Per core: x^T shard [2048, 512] bf16 (1 MB), W^T half [2048, 1024] bf16
(4 MB), out [1024, 512] fp32 (2 MB).

Device loop: k-outer over 16 contraction chunks of 128; per chunk one
256 KB W DMA + one 128 KB x DMA, then 8 back-to-back matmuls accumulating
into all 8 PSUM banks (one per 128-wide output chunk).  PE stays warm the
whole kernel; DMA (~17 us) hides under PE (~27 us).
"""

import os

import numpy as np

BATCH, SEQ = 4, 512
IN_F = OUT_F = 2048
B = 16
R = 128  # output block rows
C = 128  # input block cols
KBLK = 32  # active tiles per row

TOK = BATCH * SEQ  # 2048 tokens
TOK_SHARDS = 4
OUT_SHARDS = 2
TOK_PER = TOK // TOK_SHARDS  # 512
OUT_PER = OUT_F // OUT_SHARDS  # 1024
K_CHUNKS = IN_F // 128  # 16
M_CHUNKS = OUT_PER // 128  # 8

LAST_EXEC_TIME_NS = None

_CACHE = {}


def _ensure_profile_hook():
    """Provide antenv.axon_hooks if the image lacks it, so trace=True works.

    Mirrors trn_agent_boot._ntff_profile_via_ctypes: drives NTFF capture via
    the libaxon_pjrt.so C ABI.  Also makes upload_artifacts fall back to the
    local dir when no artifact store is reachable.
    """
    import contextlib
    import ctypes
    import sys
    import types

    try:
        import antenv.axon_hooks  # noqa: F401

        return
    except ImportError:
        pass

    so_path = "/opt/axon/libaxon_pjrt.so"
    _hook = None
    if os.path.exists(so_path):
        try:
            lib = ctypes.CDLL(so_path)
            if hasattr(lib, "axon_start_nrt_profile"):
                lib.axon_start_nrt_profile.argtypes = [
                    ctypes.POINTER(ctypes.c_int64),
                    ctypes.c_size_t,
                ]
                lib.axon_start_nrt_profile.restype = ctypes.c_int64
                lib.axon_stop_nrt_profile.argtypes = [ctypes.c_char_p]
                lib.axon_stop_nrt_profile.restype = ctypes.c_int64

                @contextlib.contextmanager
                def _ntff_hook(output_dir, device_ids):
                    import jax

                    jax.devices()
                    if device_ids:
                        ids = (ctypes.c_int64 * len(device_ids))(*device_ids)
                        rc = lib.axon_start_nrt_profile(ids, len(device_ids))
                    else:
                        rc = lib.axon_start_nrt_profile(None, 0)
                    if rc != 0:
                        raise RuntimeError(f"axon_start_nrt_profile rc={rc}")
                    try:
                        yield
                    finally:
                        n = lib.axon_stop_nrt_profile(str(output_dir).encode())
                        print(f"profile: {n} file(s) -> {output_dir}", file=sys.stderr)

                _hook = _ntff_hook
        except OSError:
            pass

    mod = types.ModuleType("antenv.axon_hooks")
    mod.get_axon_ntff_profile_hook = lambda: _hook
    sys.modules["antenv.axon_hooks"] = mod

    import concourse.bass_utils as _bu

    _orig_upload = _bu.upload_artifacts

    def _safe_upload(tmpdir):
        try:
            return _orig_upload(tmpdir)
        except Exception:
            return tmpdir

    _bu.upload_artifacts = _safe_upload


def _build_nc():
    import concourse.mybir as mybir
    from concourse import bacc
    from concourse.tile import TileContext

    nc = bacc.Bacc("TRN2", target_bir_lowering=False)
    xT = nc.dram_tensor("xT", [IN_F, TOK_PER], mybir.dt.bfloat16, kind="ExternalInput")
    w = nc.dram_tensor("w", [IN_F, OUT_PER], mybir.dt.bfloat16, kind="ExternalInput")
    bias = nc.dram_tensor("bias", [OUT_PER], mybir.dt.float32, kind="ExternalInput")
    y = nc.dram_tensor("y", [OUT_PER, TOK_PER], mybir.dt.float32, kind="ExternalOutput")

    with TileContext(nc) as tc:
        with (
            tc.tile_pool(name="consts", bufs=1) as consts,
            tc.tile_pool(name="xp", bufs=3) as xp,
            tc.tile_pool(name="wp", bufs=3) as wp,
            tc.tile_pool(name="op", bufs=M_CHUNKS) as op,
            tc.tile_pool(name="ps", bufs=1, space="PSUM") as ps,
        ):
            bias_sb = consts.tile([128, M_CHUNKS], mybir.dt.float32)
            nc.sync.dma_start(bias_sb[:], bias.rearrange("(m p) -> p m", p=128))

            psums = [
                ps.tile([128, TOK_PER], mybir.dt.float32, tag=f"ps{m}", name=f"ps{m}")
                for m in range(M_CHUNKS)
            ]

            # HAM warm-up: ~3.4us of dummy matmuls while the first DMAs land,
            # so the real stream runs at 2.4 GHz from its first instruction.
            # Accumulates garbage into psums[0]; the real k=0 matmul uses
            # start=True which resets the bank.
            warm = consts.tile([128, TOK_PER], mybir.dt.bfloat16)
            nc.gpsimd.memset(warm[:], 0)
            for i in range(8):
                nc.tensor.matmul(
                    psums[0][:],
                    warm[:, :128],
                    warm[:],
                    start=(i == 0),
                    stop=(i == 7),
                )

            for k in range(K_CHUNKS):
                xk = xp.tile([128, TOK_PER], mybir.dt.bfloat16)
                nc.sync.dma_start(xk[:], xT[k * 128 : (k + 1) * 128, :])
                wk = wp.tile([128, OUT_PER], mybir.dt.bfloat16)
                nc.sync.dma_start(wk[:], w[k * 128 : (k + 1) * 128, :])
                for m in range(M_CHUNKS):
                    nc.tensor.matmul(
                        psums[m][:],
                        wk[:, m * 128 : (m + 1) * 128],
                        xk[:],
                        start=(k == 0),
                        stop=(k == K_CHUNKS - 1),
                    )

            for m in range(M_CHUNKS):
                out_sb = op.tile([128, TOK_PER], mybir.dt.float32)
                if m % 2 == 0:
                    nc.vector.tensor_scalar_add(
                        out_sb[:], psums[m][:], bias_sb[:, m : m + 1]
                    )
                else:
                    nc.scalar.activation(
                        out_sb[:],
                        psums[m][:],
                        mybir.ActivationFunctionType.Identity,
                        bias=bias_sb[:, m : m + 1],
                    )
                nc.sync.dma_start(y[m * 128 : (m + 1) * 128, :], out_sb[:])

    nc.finalize()
    return nc


def _densify_wT(values: np.ndarray, col_indices: np.ndarray) -> np.ndarray:
    """W^T [in=2048, out=2048] with W[r*16+i, c*16+j] = values[r,k,i,j]."""
    wT = np.zeros((C, B, R, B), dtype=np.float32)  # [c, j, r, i]
    vals_t = values.transpose(0, 1, 3, 2)  # [R, K, j, i]
    r_idx = np.arange(R)
    wT[col_indices, :, r_idx[:, None], :] = vals_t
    return wT.reshape(IN_F, OUT_F)


def kernel(x, values, col_indices, bias):
    global LAST_EXEC_TIME_NS
    import ml_dtypes

    _ensure_profile_hook()
    from concourse.bass_utils import run_bass_kernel_spmd

    if "nc" not in _CACHE:
        _CACHE["nc"] = _build_nc()
    nc = _CACHE["nc"]

    bf16 = ml_dtypes.bfloat16
    wT = _densify_wT(np.asarray(values), np.asarray(col_indices)).astype(bf16)
    xT = np.ascontiguousarray(
        np.asarray(x, dtype=np.float32).reshape(TOK, IN_F).T
    ).astype(bf16)
    bias_f = np.asarray(bias, dtype=np.float32)

    in_maps = []
    for core in range(8):
        t, h = divmod(core, OUT_SHARDS)
        in_maps.append(
            {
                "xT": np.ascontiguousarray(xT[:, t * TOK_PER : (t + 1) * TOK_PER]),
                "w": np.ascontiguousarray(wT[:, h * OUT_PER : (h + 1) * OUT_PER]),
                "bias": np.ascontiguousarray(bias_f[h * OUT_PER : (h + 1) * OUT_PER]),
            }
        )

    res = run_bass_kernel_spmd(
        nc,
        in_maps,
        list(range(8)),
        trace=bool(os.environ.get("BASS_TRACE")),
    )
    LAST_EXEC_TIME_NS = res.exec_time_ns

    y = np.empty((TOK, OUT_F), dtype=np.float32)
    for core in range(8):
        t, h = divmod(core, OUT_SHARDS)
        y[t * TOK_PER : (t + 1) * TOK_PER, h * OUT_PER : (h + 1) * OUT_PER] = res.results[
            core
        ]["y"].T
    return y.reshape(BATCH, SEQ, OUT_F)


# revision 12
# speedup vs baseline: 1.1445x; 1.0954x over previous
"""CMSBlockLinear block-ELL sparse linear forward on 8 trn2 NeuronCores.

Strategy: the block-sparse weight (R=128 x K=32 active 16x16 tiles, 25%
density) is densified on the host into W^T [2048 in, 2048 out] and cast to
bf16.  The device then runs a dense matmul y^T = W^T.T @ x^T with fp32 PSUM
accumulation.  Dense-ifying costs 4x the weight FLOPs on paper, but the PE
streams N columns per matmul regardless of M, so a dense 128-wide M uses the
array 8x better than the natural M=16 sparse formulation - dense wins on both
PE time and (with bf16) roughly matches sparse fp32 on DMA bytes.

Sharding (8 cores): 4-way over tokens x 2-way over output features.
Per core: x^T shard [2048, 512] bf16 (1 MB), W^T half [2048, 1024] bf16
(4 MB), out [1024, 512] fp32 (2 MB).

Device loop: k-outer over 16 contraction chunks of 128; per chunk one
256 KB W DMA + one 128 KB x DMA, then 8 back-to-back matmuls accumulating
into all 8 PSUM banks (one per 128-wide output chunk).  PE stays warm the
whole kernel; DMA (~17 us) hides under PE (~27 us).
"""

import os

import numpy as np

BATCH, SEQ = 4, 512
IN_F = OUT_F = 2048
B = 16
R = 128  # output block rows
C = 128  # input block cols
KBLK = 32  # active tiles per row

TOK = BATCH * SEQ  # 2048 tokens
TOK_SHARDS = 4
OUT_SHARDS = 2
TOK_PER = TOK // TOK_SHARDS  # 512
OUT_PER = OUT_F // OUT_SHARDS  # 1024
K_CHUNKS = IN_F // 128  # 16
M_CHUNKS = OUT_PER // 128  # 8

LAST_EXEC_TIME_NS = None

_CACHE = {}


def _ensure_profile_hook():
    """Provide antenv.axon_hooks if the image lacks it, so trace=True works.

    Mirrors trn_agent_boot._ntff_profile_via_ctypes: drives NTFF capture via
    the libaxon_pjrt.so C ABI.  Also makes upload_artifacts fall back to the
    local dir when no artifact store is reachable.
    """
    import contextlib
    import ctypes
    import sys
    import types

    try:
        import antenv.axon_hooks  # noqa: F401

        return
    except ImportError:
        pass

    so_path = "/opt/axon/libaxon_pjrt.so"
    _hook = None
    if os.path.exists(so_path):
        try:
            lib = ctypes.CDLL(so_path)
            if hasattr(lib, "axon_start_nrt_profile"):
                lib.axon_start_nrt_profile.argtypes = [
                    ctypes.POINTER(ctypes.c_int64),
                    ctypes.c_size_t,
                ]
                lib.axon_start_nrt_profile.restype = ctypes.c_int64
                lib.axon_stop_nrt_profile.argtypes = [ctypes.c_char_p]
                lib.axon_stop_nrt_profile.restype = ctypes.c_int64

                @contextlib.contextmanager
                def _ntff_hook(output_dir, device_ids):
                    import jax

                    jax.devices()
                    if device_ids:
                        ids = (ctypes.c_int64 * len(device_ids))(*device_ids)
                        rc = lib.axon_start_nrt_profile(ids, len(device_ids))
                    else:
                        rc = lib.axon_start_nrt_profile(None, 0)
                    if rc != 0:
                        raise RuntimeError(f"axon_start_nrt_profile rc={rc}")
                    try:
                        yield
                    finally:
                        n = lib.axon_stop_nrt_profile(str(output_dir).encode())
                        print(f"profile: {n} file(s) -> {output_dir}", file=sys.stderr)

                _hook = _ntff_hook
        except OSError:
            pass

    mod = types.ModuleType("antenv.axon_hooks")
    mod.get_axon_ntff_profile_hook = lambda: _hook
    sys.modules["antenv.axon_hooks"] = mod

    import concourse.bass_utils as _bu

    _orig_upload = _bu.upload_artifacts

    def _safe_upload(tmpdir):
        try:
            return _orig_upload(tmpdir)
        except Exception:
            return tmpdir

    _bu.upload_artifacts = _safe_upload


def _build_nc():
    import concourse.mybir as mybir
    from concourse import bacc
    from concourse.tile import TileContext

    nc = bacc.Bacc("TRN2", target_bir_lowering=False)
    xT = nc.dram_tensor("xT", [IN_F, TOK_PER], mybir.dt.bfloat16, kind="ExternalInput")
    w = nc.dram_tensor("w", [IN_F, OUT_PER], mybir.dt.bfloat16, kind="ExternalInput")
    bias = nc.dram_tensor("bias", [OUT_PER], mybir.dt.float32, kind="ExternalInput")
    y = nc.dram_tensor("y", [OUT_PER, TOK_PER], mybir.dt.float32, kind="ExternalOutput")

    with TileContext(nc) as tc:
        with (
            tc.tile_pool(name="consts", bufs=1) as consts,
            tc.tile_pool(name="xp", bufs=5) as xp,
            tc.tile_pool(name="wp", bufs=5) as wp,
            tc.tile_pool(name="op", bufs=M_CHUNKS) as op,
            tc.tile_pool(name="ps", bufs=1, space="PSUM") as ps,
        ):
            psums = [
                ps.tile([128, TOK_PER], mybir.dt.float32, tag=f"ps{m}", name=f"ps{m}")
                for m in range(M_CHUNKS)
            ]

            # HAM warm-up: ~3.4us of dummy matmuls while the first DMAs land,
            # so the real stream runs at 2.4 GHz from its first instruction.
            # Accumulates garbage into psums[0]; the real k=0 matmul uses
            # start=True which resets the bank.
            warm = consts.tile([128, TOK_PER], mybir.dt.bfloat16)
            nc.vector.memset(warm[:], 0)
            for i in range(8):
                nc.tensor.matmul(
                    psums[0][:],
                    warm[:, :128],
                    warm[:],
                    start=(i == 0),
                    stop=(i == 7),
                )

            # x pushes on Sync, w pushes on GpSimd: descriptor-push is ~600ns
            # of engine time per dma_start, so serializing all of them on one
            # engine delays the first chunk (and the steady-state refill).
            xks, wks = [], []
            for k in range(K_CHUNKS):
                xk = xp.tile([128, TOK_PER], mybir.dt.bfloat16, name=f"xk{k}", tag="xk")
                nc.sync.dma_start(xk[:], xT[k * 128 : (k + 1) * 128, :])
                wk = wp.tile([128, OUT_PER], mybir.dt.bfloat16, name=f"wk{k}", tag="wk")
                nc.gpsimd.dma_start(wk[:], w[k * 128 : (k + 1) * 128, :])
                xks.append(xk)
                wks.append(wk)

            bias_sb = consts.tile([128, M_CHUNKS], mybir.dt.float32)
            nc.sync.dma_start(bias_sb[:], bias.rearrange("(m p) -> p m", p=128))

            for k in range(K_CHUNKS):
                xk, wk = xks[k], wks[k]
                for m in range(M_CHUNKS):
                    nc.tensor.matmul(
                        psums[m][:],
                        wk[:, m * 128 : (m + 1) * 128],
                        xk[:],
                        start=(k == 0),
                        stop=(k == K_CHUNKS - 1),
                    )

            for m in range(M_CHUNKS):
                out_sb = op.tile([128, TOK_PER], mybir.dt.float32)
                if m % 2 == 0:
                    nc.vector.tensor_scalar_add(
                        out_sb[:], psums[m][:], bias_sb[:, m : m + 1]
                    )
                else:
                    nc.scalar.activation(
                        out_sb[:],
                        psums[m][:],
                        mybir.ActivationFunctionType.Identity,
                        bias=bias_sb[:, m : m + 1],
                    )
                eng = nc.sync if m % 2 == 0 else nc.gpsimd
                eng.dma_start(y[m * 128 : (m + 1) * 128, :], out_sb[:])

    nc.finalize()
    return nc


def _densify_wT(values: np.ndarray, col_indices: np.ndarray) -> np.ndarray:
    """W^T [in=2048, out=2048] with W[r*16+i, c*16+j] = values[r,k,i,j]."""
    wT = np.zeros((C, B, R, B), dtype=np.float32)  # [c, j, r, i]
    vals_t = values.transpose(0, 1, 3, 2)  # [R, K, j, i]
    r_idx = np.arange(R)
    wT[col_indices, :, r_idx[:, None], :] = vals_t
    return wT.reshape(IN_F, OUT_F)


def kernel(x, values, col_indices, bias):
    global LAST_EXEC_TIME_NS
    import ml_dtypes

    _ensure_profile_hook()
    from concourse.bass_utils import run_bass_kernel_spmd

    if "nc" not in _CACHE:
        _CACHE["nc"] = _build_nc()
    nc = _CACHE["nc"]

    bf16 = ml_dtypes.bfloat16
    wT = _densify_wT(np.asarray(values), np.asarray(col_indices)).astype(bf16)
    xT = np.ascontiguousarray(
        np.asarray(x, dtype=np.float32).reshape(TOK, IN_F).T
    ).astype(bf16)
    bias_f = np.asarray(bias, dtype=np.float32)

    in_maps = []
    for core in range(8):
        t, h = divmod(core, OUT_SHARDS)
        in_maps.append(
            {
                "xT": np.ascontiguousarray(xT[:, t * TOK_PER : (t + 1) * TOK_PER]),
                "w": np.ascontiguousarray(wT[:, h * OUT_PER : (h + 1) * OUT_PER]),
                "bias": np.ascontiguousarray(bias_f[h * OUT_PER : (h + 1) * OUT_PER]),
            }
        )

    res = run_bass_kernel_spmd(
        nc,
        in_maps,
        list(range(8)),
        trace=bool(os.environ.get("BASS_TRACE")),
    )
    LAST_EXEC_TIME_NS = res.exec_time_ns

    y = np.empty((TOK, OUT_F), dtype=np.float32)
    for core in range(8):
        t, h = divmod(core, OUT_SHARDS)
        y[t * TOK_PER : (t + 1) * TOK_PER, h * OUT_PER : (h + 1) * OUT_PER] = res.results[
            core
        ]["y"].T
    return y.reshape(BATCH, SEQ, OUT_F)


# revision 18
# speedup vs baseline: 1.1789x; 1.0301x over previous
"""CMSBlockLinear block-ELL sparse linear forward on 8 trn2 NeuronCores.

Strategy: the block-sparse weight (R=128 x K=32 active 16x16 tiles, 25%
density) is densified on the host into W^T [2048 in, 2048 out] and cast to
bf16.  The device then runs a dense matmul y^T = W^T.T @ x^T with fp32 PSUM
accumulation.  Dense-ifying costs 4x the weight FLOPs on paper, but the PE
streams N columns per matmul regardless of M, so a dense 128-wide M uses the
array 8x better than the natural M=16 sparse formulation - dense wins on both
PE time and (with bf16) roughly matches sparse fp32 on DMA bytes.

Sharding (8 cores): 4-way over tokens x 2-way over output features.
Per core: x^T shard [2048, 512] bf16 (1 MB), W^T half [2048, 1024] bf16
(4 MB), out [1024, 512] fp32 (2 MB).

Device loop: k-outer over 16 contraction chunks of 128; per chunk one
256 KB W DMA + one 128 KB x DMA, then 8 back-to-back matmuls accumulating
into all 8 PSUM banks (one per 128-wide output chunk).  PE stays warm the
whole kernel; DMA (~17 us) hides under PE (~27 us).
"""

import os

import numpy as np

BATCH, SEQ = 4, 512
IN_F = OUT_F = 2048
B = 16
R = 128  # output block rows
C = 128  # input block cols
KBLK = 32  # active tiles per row

TOK = BATCH * SEQ  # 2048 tokens
TOK_SHARDS = 4
OUT_SHARDS = 2
TOK_PER = TOK // TOK_SHARDS  # 512
OUT_PER = OUT_F // OUT_SHARDS  # 1024
K_CHUNKS = IN_F // 128  # 16
M_CHUNKS = OUT_PER // 128  # 8

LAST_EXEC_TIME_NS = None

_CACHE = {}


def _ensure_profile_hook():
    """Provide antenv.axon_hooks if the image lacks it, so trace=True works.

    Mirrors trn_agent_boot._ntff_profile_via_ctypes: drives NTFF capture via
    the libaxon_pjrt.so C ABI.  Also makes upload_artifacts fall back to the
    local dir when no artifact store is reachable.
    """
    import contextlib
    import ctypes
    import sys
    import types

    try:
        import antenv.axon_hooks  # noqa: F401

        return
    except ImportError:
        pass

    so_path = "/opt/axon/libaxon_pjrt.so"
    _hook = None
    if os.path.exists(so_path):
        try:
            lib = ctypes.CDLL(so_path)
            if hasattr(lib, "axon_start_nrt_profile"):
                lib.axon_start_nrt_profile.argtypes = [
                    ctypes.POINTER(ctypes.c_int64),
                    ctypes.c_size_t,
                ]
                lib.axon_start_nrt_profile.restype = ctypes.c_int64
                lib.axon_stop_nrt_profile.argtypes = [ctypes.c_char_p]
                lib.axon_stop_nrt_profile.restype = ctypes.c_int64

                @contextlib.contextmanager
                def _ntff_hook(output_dir, device_ids):
                    import jax

                    jax.devices()
                    if device_ids:
                        ids = (ctypes.c_int64 * len(device_ids))(*device_ids)
                        rc = lib.axon_start_nrt_profile(ids, len(device_ids))
                    else:
                        rc = lib.axon_start_nrt_profile(None, 0)
                    if rc != 0:
                        raise RuntimeError(f"axon_start_nrt_profile rc={rc}")
                    try:
                        yield
                    finally:
                        n = lib.axon_stop_nrt_profile(str(output_dir).encode())
                        print(f"profile: {n} file(s) -> {output_dir}", file=sys.stderr)

                _hook = _ntff_hook
        except OSError:
            pass

    mod = types.ModuleType("antenv.axon_hooks")
    mod.get_axon_ntff_profile_hook = lambda: _hook
    sys.modules["antenv.axon_hooks"] = mod

    import concourse.bass_utils as _bu

    _orig_upload = _bu.upload_artifacts

    def _safe_upload(tmpdir):
        try:
            return _orig_upload(tmpdir)
        except Exception:
            return tmpdir

    _bu.upload_artifacts = _safe_upload


def _build_nc():
    import concourse.mybir as mybir
    from concourse import bacc
    from concourse.tile import TileContext

    nc = bacc.Bacc("TRN2", target_bir_lowering=False)
    xT = nc.dram_tensor("xT", [IN_F, TOK_PER], mybir.dt.bfloat16, kind="ExternalInput")
    w = nc.dram_tensor("w", [IN_F, OUT_PER], mybir.dt.bfloat16, kind="ExternalInput")
    bias = nc.dram_tensor("bias", [OUT_PER], mybir.dt.float32, kind="ExternalInput")
    y = nc.dram_tensor(
        "y", [OUT_PER, TOK_PER], mybir.dt.bfloat16, kind="ExternalOutput"
    )

    with TileContext(nc) as tc:
        with (
            tc.tile_pool(name="consts", bufs=1) as consts,
            tc.tile_pool(name="xp", bufs=6) as xp,
            tc.tile_pool(name="wp", bufs=6) as wp,
            tc.tile_pool(name="op", bufs=M_CHUNKS) as op,
            tc.tile_pool(name="ps", bufs=1, space="PSUM") as ps,
        ):
            psums = [
                ps.tile([128, TOK_PER], mybir.dt.float32, tag=f"ps{m}", name=f"ps{m}")
                for m in range(M_CHUNKS)
            ]

            # HAM warm-up: ~3.4us of dummy matmuls while the first DMAs land,
            # so the real stream runs at 2.4 GHz from its first instruction.
            # The warm tile's contents are irrelevant (the real k=0 matmul
            # resets psums[0] via start=True), but Tile needs a writer to
            # allocate it — one cheap column memset suffices.
            warm = consts.tile([128, TOK_PER], mybir.dt.bfloat16)
            nc.vector.memset(warm[:, :1], 0)
            for i in range(8):
                nc.tensor.matmul(
                    psums[0][:],
                    warm[:, :128],
                    warm[:],
                    start=(i == 0),
                    stop=(i == 7),
                )

            # x pushes on Sync, w pushes on GpSimd: descriptor-push is ~600ns
            # of engine time per dma_start, so serializing all of them on one
            # engine delays the first chunk (and the steady-state refill).
            xks, wks = [], []
            for k in range(K_CHUNKS):
                xk = xp.tile([128, TOK_PER], mybir.dt.bfloat16, name=f"xk{k}", tag="xk")
                nc.sync.dma_start(xk[:], xT[k * 128 : (k + 1) * 128, :])
                wk = wp.tile([128, OUT_PER], mybir.dt.bfloat16, name=f"wk{k}", tag="wk")
                nc.gpsimd.dma_start(wk[:], w[k * 128 : (k + 1) * 128, :])
                xks.append(xk)
                wks.append(wk)

            bias_sb = consts.tile([128, M_CHUNKS], mybir.dt.float32)
            nc.sync.dma_start(bias_sb[:], bias.rearrange("(m p) -> p m", p=128))

            for k in range(K_CHUNKS):
                xk, wk = xks[k], wks[k]
                for m in range(M_CHUNKS):
                    nc.tensor.matmul(
                        psums[m][:],
                        wk[:, m * 128 : (m + 1) * 128],
                        xk[:],
                        start=(k == 0),
                        stop=(k == K_CHUNKS - 1),
                    )

            for m in range(M_CHUNKS):
                out_sb = op.tile([128, TOK_PER], mybir.dt.bfloat16)
                if m % 2 == 0:
                    nc.vector.tensor_scalar_add(
                        out_sb[:], psums[m][:], bias_sb[:, m : m + 1]
                    )
                else:
                    nc.scalar.activation(
                        out_sb[:],
                        psums[m][:],
                        mybir.ActivationFunctionType.Identity,
                        bias=bias_sb[:, m : m + 1],
                    )
                eng = nc.sync if m % 2 == 0 else nc.gpsimd
                eng.dma_start(y[m * 128 : (m + 1) * 128, :], out_sb[:])

    nc.finalize()
    return nc


def _densify_wT(values: np.ndarray, col_indices: np.ndarray) -> np.ndarray:
    """W^T [in=2048, out=2048] with W[r*16+i, c*16+j] = values[r,k,i,j]."""
    wT = np.zeros((C, B, R, B), dtype=np.float32)  # [c, j, r, i]
    vals_t = values.transpose(0, 1, 3, 2)  # [R, K, j, i]
    r_idx = np.arange(R)
    wT[col_indices, :, r_idx[:, None], :] = vals_t
    return wT.reshape(IN_F, OUT_F)


def kernel(x, values, col_indices, bias):
    global LAST_EXEC_TIME_NS
    import ml_dtypes

    _ensure_profile_hook()
    from concourse.bass_utils import run_bass_kernel_spmd

    if "nc" not in _CACHE:
        _CACHE["nc"] = _build_nc()
    nc = _CACHE["nc"]

    bf16 = ml_dtypes.bfloat16
    wT = _densify_wT(np.asarray(values), np.asarray(col_indices)).astype(bf16)
    xT = np.ascontiguousarray(
        np.asarray(x, dtype=np.float32).reshape(TOK, IN_F).T
    ).astype(bf16)
    bias_f = np.asarray(bias, dtype=np.float32)

    in_maps = []
    for core in range(8):
        t, h = divmod(core, OUT_SHARDS)
        in_maps.append(
            {
                "xT": np.ascontiguousarray(xT[:, t * TOK_PER : (t + 1) * TOK_PER]),
                "w": np.ascontiguousarray(wT[:, h * OUT_PER : (h + 1) * OUT_PER]),
                "bias": np.ascontiguousarray(bias_f[h * OUT_PER : (h + 1) * OUT_PER]),
            }
        )

    res = run_bass_kernel_spmd(
        nc,
        in_maps,
        list(range(8)),
        trace=bool(os.environ.get("BASS_TRACE")),
    )
    LAST_EXEC_TIME_NS = res.exec_time_ns

    y = np.empty((TOK, OUT_F), dtype=np.float32)
    for core in range(8):
        t, h = divmod(core, OUT_SHARDS)
        y[t * TOK_PER : (t + 1) * TOK_PER, h * OUT_PER : (h + 1) * OUT_PER] = (
            res.results[core]["y"].astype(np.float32).T
        )
    return y.reshape(BATCH, SEQ, OUT_F)
